# revision 25
# baseline (speedup 1.0000x reference)
"""AgentAttention TRN2 kernel (v4).

Math (per batch b, head h):
  q,k,v = split_heads(x @ w_qkv.T)                    # (n, d) each, d=64
  qa  = softmax(q @ agent_h.T * scale, axis=m)        # (n, m), m=256
  ak  = softmax(agent_h @ k.T, axis=n)                # (m, n)
  kv  = softmax(ak @ v, axis=d)                       # (m, d)
  out = qa @ kv                                       # (n, d)

Softmax trick: softmax(X) @ Y == (exp(X) @ [Y, 1]) -> divide by last col.
Sharding: 8 cores = 4 batches x 2 head-groups (4 heads each).

v4 (on top of v3's host-transposed fp16 x and all-heads pass-A e1):
 - All pass-A logits (qk, v, E1, E2) share ONE 3-slot pool of 2-bank
   psum tiles; v-proj no longer has its own pool.  3 slots instead of 2
   un-serializes the matmul->exp/copy pipeline that paced v3.
 - Host packs x/wqk/wv partition-major so every DMA descriptor is
   2-4KB contiguous (v3's 1KB rows made the initial weight DMA ~10us).
 - Out-stage matmuls col-tiled into concurrent 64-token pairs
   (LDWEIGHTS was 92ns vs 27ns matmul; halved+overlapped loads).
 - Pass-B normalization split DVE/ACT by token-tile parity (ACT is
   otherwise idle in pass B).

Dtypes: fp16 x/w/q/k/agent/e1/kv_aug (exp(s1*scale)<=~1.7e3 fits fp16);
bf16 e2/v (exp(s2) up to ~4e25 needs bf16 range; softmax-damped).
"""
import sys
import os

sys.path.insert(0, "/opt/trn_rl_repo")

import numpy as np

HEADS = 8
D = 64              # dim per head
M = 256             # agent tokens
DIM = 512
N = 8192            # sequence length
B = 4
SCALE = D ** -0.5
ST = 512            # pass-A token super-tile
NST = N // ST       # 16

_cached = {}


def _build():
    import concourse.bass as bass
    import concourse.bacc as bacc
    import concourse.tile as tile
    from concourse import mybir, masks
    from contextlib import ExitStack

    f32 = mybir.dt.float32
    bf16 = mybir.dt.bfloat16
    fp16 = mybir.dt.float16
    i16 = mybir.dt.int16
    EXP = mybir.ActivationFunctionType.Exp

    nc = bacc.Bacc("TRN2", target_bir_lowering=False, debug=False)

    # x: host-transposed/cast/packed: row (st*128+p) = [4ci x 512t] fp16
    x_ap = nc.dram_tensor("x", [NST * 128, 4 * ST], fp16, kind="ExternalInput").ap()
    # wqk: host-packed [128, 4ci x 512cols]; wv: [128, 4ci x 256cols]
    wqk_ap = nc.dram_tensor("wqk", [128, 2048], fp16, kind="ExternalInput").ap()
    wv_ap = nc.dram_tensor("wv", [128, 1024], fp16, kind="ExternalInput").ap()
    ag_ap = nc.dram_tensor("ag", [128, 1024], fp16, kind="ExternalInput").ap()
    out_ap = nc.dram_tensor("out", [N, 256], f32, kind="ExternalOutput").ap()

    with tile.TileContext(nc) as tc, ExitStack() as ctx:
        const = ctx.enter_context(tc.tile_pool(name="const", bufs=1))
        e1pool = ctx.enter_context(tc.tile_pool(name="e1pool", bufs=1))

        ident = const.tile([128, 128], f32, tag="ident")
        masks.make_identity(nc, ident[:])

        wqk_h = const.tile([128, 4, 512], fp16, tag="wqk_h")
        # split by head-pair columns: the first qk matmul group only needs
        # hp=0's 256 columns, so it can start ~2-4us sooner
        wqk_v = wqk_ap.rearrange("p (ci w) -> p ci w", ci=4)
        nc.sync.dma_start(wqk_h[:, :, 0:256], wqk_v[:, :, 0:256])
        nc.sync.dma_start(wqk_h[:, :, 256:512], wqk_v[:, :, 256:512])
        wv_h = const.tile([128, 4, 256], fp16, tag="wv_h")
        nc.sync.dma_start(wv_h[:], wv_ap.rearrange("p (ci w) -> p ci w", ci=4))
        ag_h = const.tile([128, 1024], fp16, tag="ag_h")
        nc.sync.dma_start(ag_h[:], ag_ap[:])

        with tc.tile_pool(name="stage", bufs=1) as stage:
            ones_s = stage.tile([128, 16], f32, tag="ones_s")
            nc.gpsimd.memset(ones_s[:], 1.0)
            ones_b = const.tile([128, 4, 4], bf16, tag="ones_b")
            nc.vector.tensor_copy(ones_b[:], ones_s[:].rearrange("p (a b) -> p a b", a=4))
            ones_h = const.tile([128, 4], fp16, tag="ones_h")
            nc.vector.tensor_copy(ones_h[:], ones_s[:, 0:4])

        # e1 for all heads, persisted through pass B.
        # e1s[hp][mc][:, jj, t] = exp(scale * q_{2hp+jj} @ agent^T)[mc*128+p, t]
        e1s = [[e1pool.tile([128, 2, N], fp16, tag=f"e1s{hp}_{mc}",
                            name=f"e1s{hp}_{mc}")
                for mc in range(2)] for hp in range(2)]

        # KV_aug per head per m-chunk: (128, 66) fp16
        kv_aug = [[const.tile([128, 66], fp16, tag=f"kva{j}_{mc}",
                              name=f"kva{j}_{mc}")
                   for mc in range(2)] for j in range(4)]

        # ================= PASS A =================
        with ExitStack() as actx:
            xtp = actx.enter_context(tc.tile_pool(name="xtp", bufs=3))
            qkp = actx.enter_context(tc.tile_pool(name="qkp", bufs=4))
            e2tp = actx.enter_context(tc.tile_pool(name="e2tp", bufs=5))
            vp = actx.enter_context(tc.tile_pool(name="vp", bufs=4))
            schp = actx.enter_context(tc.tile_pool(name="schp", bufs=3))
            # PSUM (8 banks): pbig [128,2,512] f32 x3 = 6 banks
            #                 kvp persistent [65,2,256] f32 x2 = 2 banks
            pbig = actx.enter_context(tc.tile_pool(name="pbig", bufs=3, space="PSUM"))
            pkvp = actx.enter_context(tc.tile_pool(name="pkvp", bufs=1, space="PSUM"))

            # persistent KV accumulators: kvp[hp][:, jj, m] for head 2hp+jj
            # rows 0-63 = sum_n e2[n,m]*v[n,d], row 64 = sum_n e2[n,m]
            kvp = [pkvp.tile([65, 2, 256], f32, tag=f"kvp{hp}", name=f"kvp{hp}")
                   for hp in range(2)]

            for st in range(NST):
                r0 = st * ST
                xT = xtp.tile([128, 4, ST], fp16, tag="xT")
                nc.sync.dma_start(
                    xT[:], x_ap[st * 128:(st + 1) * 128, :]
                    .rearrange("p (ci t) -> p ci t", ci=4))

                # qk projection per head-pair into one pbig tile:
                # [:, 0, :] = qT rows [qA|qB], [:, 1, :] = kT rows [kA|kB]
                qkT = {}
                for hp in range(2):
                    pq = pbig.tile([128, 2, 512], f32, tag="pbig",
                                   name=f"pqk{st}_{hp}")
                    for qk in range(2):
                        for ci in range(4):
                            nc.tensor.matmul(
                                pq[:, qk, :],
                                wqk_h[:, ci, hp * 256 + qk * 128:
                                      hp * 256 + (qk + 1) * 128],
                                xT[:, ci, :],
                                start=(ci == 0), stop=(ci == 3))
                    qt = qkp.tile([128, 2, ST], fp16, tag="qkT",
                                  name=f"qkT{st}_{hp}")
                    # split q/k halves so the q half (E1's operand) is
                    # available one DVE-op earlier than the k half
                    nc.vector.tensor_copy(qt[:, 0, :], pq[:, 0, :])
                    nc.vector.tensor_copy(qt[:, 1, :], pq[:, 1, :])
                    qkT[hp] = qt

                # v projection: all 4 token-subtiles into one pbig tile
                # (PE work that overlaps the DVE qk copies)
                pv = pbig.tile([128, 2, 512], f32, tag="pbig",
                               name=f"pv{st}")
                for tt in range(4):
                    for ci in range(4):
                        nc.tensor.matmul(
                            pv[:, tt // 2, (tt % 2) * 256:(tt % 2 + 1) * 256],
                            xT[:, ci, tt * 128:(tt + 1) * 128],
                            wv_h[:, ci, :],
                            start=(ci == 0), stop=(ci == 3))
                vt = vp.tile([128, 4, 4, 65], bf16, tag="v_t",
                             name=f"v{st}")
                nc.vector.tensor_copy(
                    vt[:, :, :, 0:64],
                    pv[:].rearrange("p h (s j d) -> p (h s) j d", j=4, s=2))
                nc.vector.tensor_copy(vt[:, :, :, 64], ones_b[:])

                # E1 = exp(scale * q @ agT) for all 4 heads; head pairs run
                # as concurrent row-tiled matmuls into one 2-bank psum tile.
                # hp=0 exps on ACT; hp=1 exps on DVE via the Schraudolph
                # bit-trick: exp(x) ~= bitcast_fp16(int16(A*x + B)) with
                # A = 2^10/ln2 * SCALE, B = 15*2^10 - 7.42 (max rel err ~3%,
                # damped to ~0.3% by the diffuse qa softmax).
                def e1_tile(hp, mc):
                    pe = pbig.tile([128, 2, 512], f32, tag="pbig",
                                   name=f"pE{st}_{hp}_{mc}")
                    for jj in range(2):
                        rb = jj * 64
                        j = 2 * hp + jj
                        nc.tensor.matmul(
                            pe[:, jj, :],
                            ag_h[rb:rb + 64,
                                 j * 256 + mc * 128:j * 256 + (mc + 1) * 128],
                            qkT[hp][rb:rb + 64, 0, :],
                            start=True, stop=True)
                    dst = e1s[hp][mc][:, :, r0:r0 + ST]
                    if not (hp == 1 and mc == 1):
                        nc.scalar.activation(dst, pe[:], EXP, scale=SCALE)
                    else:
                        tmp = schp.tile([128, 2, 512], f32, tag="sch",
                                        name=f"sch{st}_{mc}")
                        nc.vector.tensor_scalar(
                            tmp[:], pe[:], 184.664953, 15315.25,
                            mybir.AluOpType.mult, mybir.AluOpType.add)
                        nc.vector.tensor_copy(dst.bitcast(i16), tmp[:])

                # E2 = exp(k @ agT), then KV accumulate (lagged one tile so
                # KV matmuls never head-of-line-block on the exp)
                def kv_mms(hp_, half_, e2t_):
                    # One psum group per kvp bank: start only on the very
                    # first matmul (its zero-region marking makes jj=1's
                    # first write an overwrite too), stop only on the last.
                    for s_ in range(2):
                        tt_ = half_ * 2 + s_
                        for jj_ in range(2):
                            first = (st == 0 and tt_ == 0 and jj_ == 0)
                            last = (st == NST - 1 and tt_ == 3 and jj_ == 1)
                            nc.tensor.matmul(
                                kvp[hp_][:, jj_, :],
                                vt[:, tt_, 2 * hp_ + jj_, :],
                                e2t_[:, jj_, s_ * 256:(s_ + 1) * 256],
                                start=first, stop=last)

                def e2_tile(hp, half):
                    pe2 = pbig.tile([128, 2, 512], f32, tag="pbig",
                                    name=f"pe2{st}_{hp}_{half}")
                    for s in range(2):
                        tt = half * 2 + s
                        for jj in range(2):
                            rb = jj * 64
                            j = 2 * hp + jj
                            nc.tensor.matmul(
                                pe2[:, jj, s * 256:(s + 1) * 256],
                                qkT[hp][rb:rb + 64, 1,
                                        tt * 128:(tt + 1) * 128],
                                ag_h[rb:rb + 64, j * 256:(j + 1) * 256],
                                start=True, stop=True)
                    e2t = e2tp.tile([128, 2, 512], bf16, tag="e2t",
                                    name=f"e2t{st}_{hp}_{half}")
                    nc.scalar.activation(e2t[:], pe2[:], EXP)
                    return e2t

                # interleave E2 (ACT, gates KV) ahead of E1 (ACT/DVE, gates
                # nothing until pass B) in each round; KV matmuls lag their
                # exp by three tiles for sem-latency slack
                e1_order = [(0, 0), (1, 1), (0, 1), (1, 0)]
                e2_order = [(0, 0), (0, 1), (1, 0), (1, 1)]
                pend = []
                for idx in range(4):
                    e2t = e2_tile(*e2_order[idx])
                    pend.append((e2_order[idx][0], e2_order[idx][1], e2t))
                    e1_tile(*e1_order[idx])
                    if len(pend) > 2:
                        kv_mms(*pend.pop(0))
                for p in pend:
                    kv_mms(*p)

            # ---- kv finalize per head ----
            fin = actx.enter_context(tc.tile_pool(name="fin", bufs=1))
            kvsb = []
            for hp in range(2):
                t = fin.tile([65, 2, 256], f32, tag=f"kvsb{hp}")
                nc.vector.tensor_copy(t[:], kvp[hp][:])
                kvsb.append(t)
            for j in range(4):
                hp, jj = j // 2, j % 2
                for mc in range(2):
                    pt = pbig.tile([128, 2, 512], f32, tag="pbig",
                                   name=f"pfin{j}_{mc}")
                    nc.tensor.transpose(
                        pt[:, 0, 0:65],
                        kvsb[hp][:, jj, mc * 128:(mc + 1) * 128],
                        ident[0:65, 0:65])
                    den = fin.tile([128, 1], f32, tag=f"den{j}{mc}")
                    nc.vector.reciprocal(den[:], pt[:, 0, 64:65])
                    kve = fin.tile([128, 64], f32, tag=f"kve{j}{mc}")
                    esum = fin.tile([128, 1], f32, tag=f"es{j}{mc}")
                    nc.scalar.activation(kve[:], pt[:, 0, 0:64], EXP,
                                         scale=den[:], accum_out=esum[:])
                    rsum = fin.tile([128, 1], f32, tag=f"rs{j}{mc}")
                    nc.vector.reciprocal(rsum[:], esum[:])
                    nc.vector.tensor_scalar_mul(kv_aug[j][mc][:, 0:64],
                                                kve[:], rsum[:])
                    nc.vector.tensor_copy(kv_aug[j][mc][:, 64:66],
                                          ones_h[:, 0:2])

        # ================= PASS B: out = (e1/rowsum) @ kv =================
        with ExitStack() as bctx:
            outp = bctx.enter_context(tc.tile_pool(name="outp", bufs=4))
            pout = bctx.enter_context(tc.tile_pool(name="pout", bufs=4, space="PSUM"))

            # 2 token-tiles (256 tokens) per 2-bank psum tile to halve the
            # per-iteration slot/sem overhead.
            for it in range(N // 256):
                c0 = it * 256
                # padded so the u-dim stride is one full psum bank: head j=3's
                # 264B output must not cross the 2KB bank boundary.
                po = pout.tile([128, 2, 4, 66], f32, tag="pout",
                               padded_shape=[128, 2, 4, 128])
                for u in range(2):
                    for hp in range(2):
                        for jj in range(2):
                            j = 2 * hp + jj
                            for mc in range(2):
                                # col-tiled concurrent 64-token pair;
                                # pending-zero marks are per-partition, so
                                # each half needs its own start/stop.
                                for half in range(2):
                                    t0 = c0 + u * 128 + half * 64
                                    nc.tensor.matmul(
                                        po[half * 64:(half + 1) * 64, u, j, :],
                                        e1s[hp][mc][:, jj, t0:t0 + 64],
                                        kv_aug[j][mc][:],
                                        start=(j == 0 and mc == 0),
                                        stop=(j == 3 and mc == 1))
                rec = outp.tile([128, 2, 4], f32, tag="rec")
                nc.vector.reciprocal(rec[:], po[:, :, :, 64])
                ot = outp.tile([128, 2, 4, 64], f32, tag="ot")
                nc.vector.tensor_tensor(
                    ot[:], po[:, :, :, 0:64],
                    rec[:].unsqueeze(3).broadcast_to((128, 2, 4, 64)),
                    mybir.AluOpType.mult)
                nc.sync.dma_start(
                    out_ap[c0:c0 + 256, :].rearrange("(u p) c -> p u c", u=2),
                    ot[:].rearrange("p u j d -> p u (j d)"))

    nc.compile()
    return nc


def _get_program():
    if "nc" not in _cached:
        _cached["nc"] = _build()
    return _cached["nc"]


def kernel(x, w_qkv, agent):
    from concourse.bass_utils import run_bass_kernel_spmd

    nc = _get_program()

    x = np.asarray(x, dtype=np.float32)
    w_qkv = np.asarray(w_qkv, dtype=np.float32)
    agent = np.asarray(agent, dtype=np.float32)

    in_maps = []
    for core in range(8):
        bi, hg = core // 2, core % 2
        heads = [4 * hg + jj for jj in range(4)]
        wqk = np.empty((DIM, 512), np.float16)
        for hp in range(2):
            hA, hB = heads[2 * hp], heads[2 * hp + 1]
            wqk[:, hp * 256 + 0:hp * 256 + 64] = w_qkv[hA * 64:(hA + 1) * 64, :].T
            wqk[:, hp * 256 + 64:hp * 256 + 128] = w_qkv[hB * 64:(hB + 1) * 64, :].T
            wqk[:, hp * 256 + 128:hp * 256 + 192] = \
                w_qkv[DIM + hA * 64:DIM + (hA + 1) * 64, :].T
            wqk[:, hp * 256 + 192:hp * 256 + 256] = \
                w_qkv[DIM + hB * 64:DIM + (hB + 1) * 64, :].T
        # pack [512c, 512cols] -> [128p, 4ci*512cols]
        wqk_p = np.ascontiguousarray(
            wqk.reshape(4, 128, 512).transpose(1, 0, 2).reshape(128, 2048))
        wv = np.empty((DIM, 256), np.float16)
        for jj, hh in enumerate(heads):
            wv[:, jj * 64:(jj + 1) * 64] = \
                w_qkv[2 * DIM + hh * 64:2 * DIM + (hh + 1) * 64, :].T
        wv_p = np.ascontiguousarray(
            wv.reshape(4, 128, 256).transpose(1, 0, 2).reshape(128, 1024))
        ag = np.empty((128, 1024), np.float16)
        for jj, hh in enumerate(heads):
            agT = agent[hh].T
            ag[0:64, jj * 256:(jj + 1) * 256] = agT
            ag[64:128, jj * 256:(jj + 1) * 256] = agT
        # x: [n, 512c] -> xT [512c, n] -> packed [(st p), (ci t)]
        xt = x[bi].T.astype(np.float16)                       # [512, 8192]
        xt_p = np.ascontiguousarray(
            xt.reshape(4, 128, NST, ST).transpose(2, 1, 0, 3)
            .reshape(NST * 128, 4 * ST))
        in_maps.append({"x": xt_p, "wqk": wqk_p, "wv": wv_p, "ag": ag})

    res = run_bass_kernel_spmd(nc, in_maps, core_ids=list(range(8)),
                               trace=bool(os.environ.get("AGENT_TRACE")))
    out = np.empty((B, N, DIM), np.float32)
    for core in range(8):
        bi, hg = core // 2, core % 2
        out[bi, :, hg * 256:(hg + 1) * 256] = res.results[core]["out"]
    if res.exec_time_ns is not None:
        kernel.last_exec_time_ns = res.exec_time_ns
        kernel.last_mean_exec_time_ns = res.mean_exec_time_ns
        kernel.last_trace = res.instructions_and_trace
    return out


# revision 27
# speedup vs baseline: 1.0116x; 1.0116x over previous
"""AgentAttention TRN2 kernel (v4).

Math (per batch b, head h):
  q,k,v = split_heads(x @ w_qkv.T)                    # (n, d) each, d=64
  qa  = softmax(q @ agent_h.T * scale, axis=m)        # (n, m), m=256
  ak  = softmax(agent_h @ k.T, axis=n)                # (m, n)
  kv  = softmax(ak @ v, axis=d)                       # (m, d)
  out = qa @ kv                                       # (n, d)

Softmax trick: softmax(X) @ Y == (exp(X) @ [Y, 1]) -> divide by last col.
Sharding: 8 cores = 4 batches x 2 head-groups (4 heads each).

v4 (on top of v3's host-transposed fp16 x and all-heads pass-A e1):
 - All pass-A logits (qk, v, E1, E2) share ONE 3-slot pool of 2-bank
   psum tiles; v-proj no longer has its own pool.  3 slots instead of 2
   un-serializes the matmul->exp/copy pipeline that paced v3.
 - Host packs x/wqk/wv partition-major so every DMA descriptor is
   2-4KB contiguous (v3's 1KB rows made the initial weight DMA ~10us).
 - Out-stage matmuls col-tiled into concurrent 64-token pairs
   (LDWEIGHTS was 92ns vs 27ns matmul; halved+overlapped loads).
 - Pass-B normalization split DVE/ACT by token-tile parity (ACT is
   otherwise idle in pass B).

Dtypes: fp16 x/w/q/k/agent/e1/kv_aug (exp(s1*scale)<=~1.7e3 fits fp16);
bf16 e2/v (exp(s2) up to ~4e25 needs bf16 range; softmax-damped).
"""
import sys
import os

sys.path.insert(0, "/opt/trn_rl_repo")

import numpy as np

HEADS = 8
D = 64              # dim per head
M = 256             # agent tokens
DIM = 512
N = 8192            # sequence length
B = 4
SCALE = D ** -0.5
ST = 512            # pass-A token super-tile
NST = N // ST       # 16

_cached = {}


def _build():
    import concourse.bass as bass
    import concourse.bacc as bacc
    import concourse.tile as tile
    from concourse import mybir, masks
    from contextlib import ExitStack

    f32 = mybir.dt.float32
    bf16 = mybir.dt.bfloat16
    fp16 = mybir.dt.float16
    i16 = mybir.dt.int16
    EXP = mybir.ActivationFunctionType.Exp

    nc = bacc.Bacc("TRN2", target_bir_lowering=False, debug=False)

    # x: host-transposed/cast/packed: row (st*128+p) = [4ci x 512t] fp16
    x_ap = nc.dram_tensor("x", [NST * 128, 4 * ST], fp16, kind="ExternalInput").ap()
    # wqk: host-packed [128, 4ci x 512cols]; wv: [128, 4ci x 256cols]
    wqk_ap = nc.dram_tensor("wqk", [128, 2048], fp16, kind="ExternalInput").ap()
    wv_ap = nc.dram_tensor("wv", [128, 1024], fp16, kind="ExternalInput").ap()
    ag_ap = nc.dram_tensor("ag", [128, 1024], fp16, kind="ExternalInput").ap()
    out_ap = nc.dram_tensor("out", [N, 256], f32, kind="ExternalOutput").ap()

    with tile.TileContext(nc) as tc, ExitStack() as ctx:
        const = ctx.enter_context(tc.tile_pool(name="const", bufs=1))
        e1pool = ctx.enter_context(tc.tile_pool(name="e1pool", bufs=1))

        ident = const.tile([128, 128], f32, tag="ident")
        masks.make_identity(nc, ident[:])

        wqk_h = const.tile([128, 4, 512], fp16, tag="wqk_h")
        nc.sync.dma_start(wqk_h[:], wqk_ap.rearrange("p (ci w) -> p ci w", ci=4))
        wv_h = const.tile([128, 4, 256], fp16, tag="wv_h")
        nc.sync.dma_start(wv_h[:], wv_ap.rearrange("p (ci w) -> p ci w", ci=4))
        ag_h = const.tile([128, 1024], fp16, tag="ag_h")
        nc.sync.dma_start(ag_h[:], ag_ap[:])

        with tc.tile_pool(name="stage", bufs=1) as stage:
            ones_s = stage.tile([128, 16], f32, tag="ones_s")
            nc.gpsimd.memset(ones_s[:], 1.0)
            ones_b = const.tile([128, 4, 4], bf16, tag="ones_b")
            nc.vector.tensor_copy(ones_b[:], ones_s[:].rearrange("p (a b) -> p a b", a=4))
            ones_h = const.tile([128, 4], fp16, tag="ones_h")
            nc.vector.tensor_copy(ones_h[:], ones_s[:, 0:4])

        # e1 for all heads, persisted through pass B.
        # e1s[hp][mc][:, jj, t] = exp(scale * q_{2hp+jj} @ agent^T)[mc*128+p, t]
        e1s = [[e1pool.tile([128, 2, N], fp16, tag=f"e1s{hp}_{mc}",
                            name=f"e1s{hp}_{mc}")
                for mc in range(2)] for hp in range(2)]

        # KV_aug per head per m-chunk: (128, 66) fp16
        kv_aug = [[const.tile([128, 66], fp16, tag=f"kva{j}_{mc}",
                              name=f"kva{j}_{mc}")
                   for mc in range(2)] for j in range(4)]

        # ================= PASS A =================
        with ExitStack() as actx:
            xtp = actx.enter_context(tc.tile_pool(name="xtp", bufs=3))
            qkp = actx.enter_context(tc.tile_pool(name="qkp", bufs=3))
            e2tp = actx.enter_context(tc.tile_pool(name="e2tp", bufs=5))
            vp = actx.enter_context(tc.tile_pool(name="vp", bufs=3))
            schp = actx.enter_context(tc.tile_pool(name="schp", bufs=3))
            # PSUM (8 banks): pbig [128,2,512] f32 x3 = 6 banks
            #                 kvp persistent [65,2,256] f32 x2 = 2 banks
            pbig = actx.enter_context(tc.tile_pool(name="pbig", bufs=3, space="PSUM"))
            pkvp = actx.enter_context(tc.tile_pool(name="pkvp", bufs=1, space="PSUM"))

            # persistent KV accumulators: kvp[hp][:, jj, m] for head 2hp+jj
            # rows 0-63 = sum_n e2[n,m]*v[n,d], row 64 = sum_n e2[n,m]
            kvp = [pkvp.tile([65, 2, 256], f32, tag=f"kvp{hp}", name=f"kvp{hp}")
                   for hp in range(2)]

            for st in range(NST):
                r0 = st * ST
                xT = xtp.tile([128, 4, ST], fp16, tag="xT")
                nc.sync.dma_start(
                    xT[:], x_ap[st * 128:(st + 1) * 128, :]
                    .rearrange("p (ci t) -> p ci t", ci=4))

                # qk projection per head-pair into one pbig tile:
                # [:, 0, :] = qT rows [qA|qB], [:, 1, :] = kT rows [kA|kB]
                qkT = {}
                for hp in range(2):
                    pq = pbig.tile([128, 2, 512], f32, tag="pbig",
                                   name=f"pqk{st}_{hp}")
                    for qk in range(2):
                        for ci in range(4):
                            nc.tensor.matmul(
                                pq[:, qk, :],
                                wqk_h[:, ci, hp * 256 + qk * 128:
                                      hp * 256 + (qk + 1) * 128],
                                xT[:, ci, :],
                                start=(ci == 0), stop=(ci == 3))
                    qt = qkp.tile([128, 2, ST], fp16, tag="qkT",
                                  name=f"qkT{st}_{hp}")
                    # split q/k halves so the q half (E1's operand) is
                    # available one DVE-op earlier than the k half
                    nc.vector.tensor_copy(qt[:, 0, :], pq[:, 0, :])
                    nc.vector.tensor_copy(qt[:, 1, :], pq[:, 1, :])
                    qkT[hp] = qt

                # v projection: all 4 token-subtiles into one pbig tile
                # (PE work that overlaps the DVE qk copies)
                pv = pbig.tile([128, 2, 512], f32, tag="pbig",
                               name=f"pv{st}")
                for tt in range(4):
                    for ci in range(4):
                        nc.tensor.matmul(
                            pv[:, tt // 2, (tt % 2) * 256:(tt % 2 + 1) * 256],
                            xT[:, ci, tt * 128:(tt + 1) * 128],
                            wv_h[:, ci, :],
                            start=(ci == 0), stop=(ci == 3))
                vt = vp.tile([128, 4, 4, 65], bf16, tag="v_t",
                             name=f"v{st}")
                nc.vector.tensor_copy(
                    vt[:, :, :, 0:64],
                    pv[:].rearrange("p h (s j d) -> p (h s) j d", j=4, s=2))
                nc.vector.tensor_copy(vt[:, :, :, 64], ones_b[:])

                # E1 = exp(scale * q @ agT) for all 4 heads; head pairs run
                # as concurrent row-tiled matmuls into one 2-bank psum tile.
                # hp=0 exps on ACT; hp=1 exps on DVE via the Schraudolph
                # bit-trick: exp(x) ~= bitcast_fp16(int16(A*x + B)) with
                # A = 2^10/ln2 * SCALE, B = 15*2^10 - 7.42 (max rel err ~3%,
                # damped to ~0.3% by the diffuse qa softmax).
                def e1_tile(hp, mc):
                    pe = pbig.tile([128, 2, 512], f32, tag="pbig",
                                   name=f"pE{st}_{hp}_{mc}")
                    for jj in range(2):
                        rb = jj * 64
                        j = 2 * hp + jj
                        nc.tensor.matmul(
                            pe[:, jj, :],
                            ag_h[rb:rb + 64,
                                 j * 256 + mc * 128:j * 256 + (mc + 1) * 128],
                            qkT[hp][rb:rb + 64, 0, :],
                            start=True, stop=True)
                    dst = e1s[hp][mc][:, :, r0:r0 + ST]
                    if hp == 0:
                        nc.scalar.activation(dst, pe[:], EXP, scale=SCALE)
                    else:
                        tmp = schp.tile([128, 2, 512], f32, tag="sch",
                                        name=f"sch{st}_{mc}")
                        nc.vector.tensor_scalar(
                            tmp[:], pe[:], 184.664953, 15315.25,
                            mybir.AluOpType.mult, mybir.AluOpType.add)
                        nc.vector.tensor_copy(dst.bitcast(i16), tmp[:])

                # E2 = exp(k @ agT), then KV accumulate (lagged one tile so
                # KV matmuls never head-of-line-block on the exp)
                def kv_mms(hp_, half_, e2t_):
                    # One psum group per kvp bank: start only on the very
                    # first matmul (its zero-region marking makes jj=1's
                    # first write an overwrite too), stop only on the last.
                    for s_ in range(2):
                        tt_ = half_ * 2 + s_
                        for jj_ in range(2):
                            first = (st == 0 and tt_ == 0 and jj_ == 0)
                            last = (st == NST - 1 and tt_ == 3 and jj_ == 1)
                            nc.tensor.matmul(
                                kvp[hp_][:, jj_, :],
                                vt[:, tt_, 2 * hp_ + jj_, :],
                                e2t_[:, jj_, s_ * 256:(s_ + 1) * 256],
                                start=first, stop=last)

                def e2_tile(hp, half):
                    pe2 = pbig.tile([128, 2, 512], f32, tag="pbig",
                                    name=f"pe2{st}_{hp}_{half}")
                    for s in range(2):
                        tt = half * 2 + s
                        for jj in range(2):
                            rb = jj * 64
                            j = 2 * hp + jj
                            nc.tensor.matmul(
                                pe2[:, jj, s * 256:(s + 1) * 256],
                                qkT[hp][rb:rb + 64, 1,
                                        tt * 128:(tt + 1) * 128],
                                ag_h[rb:rb + 64, j * 256:(j + 1) * 256],
                                start=True, stop=True)
                    e2t = e2tp.tile([128, 2, 512], bf16, tag="e2t",
                                    name=f"e2t{st}_{hp}_{half}")
                    nc.scalar.activation(e2t[:], pe2[:], EXP)
                    return e2t

                # interleave E2 (ACT, gates KV) ahead of E1 (ACT/DVE, gates
                # nothing until pass B) in each round; KV matmuls lag their
                # exp by three tiles for sem-latency slack
                e1_order = [(0, 0), (1, 1), (0, 1), (1, 0)]
                e2_order = [(0, 0), (0, 1), (1, 0), (1, 1)]
                pend = []
                for idx in range(4):
                    e2t = e2_tile(*e2_order[idx])
                    pend.append((e2_order[idx][0], e2_order[idx][1], e2t))
                    e1_tile(*e1_order[idx])
                    if len(pend) > 2:
                        kv_mms(*pend.pop(0))
                for p in pend:
                    kv_mms(*p)

            # ---- kv finalize per head ----
            fin = actx.enter_context(tc.tile_pool(name="fin", bufs=1))
            kvsb = []
            for hp in range(2):
                t = fin.tile([65, 2, 256], f32, tag=f"kvsb{hp}")
                nc.vector.tensor_copy(t[:], kvp[hp][:])
                kvsb.append(t)
            for j in range(4):
                hp, jj = j // 2, j % 2
                for mc in range(2):
                    pt = pbig.tile([128, 2, 512], f32, tag="pbig",
                                   name=f"pfin{j}_{mc}")
                    nc.tensor.transpose(
                        pt[:, 0, 0:65],
                        kvsb[hp][:, jj, mc * 128:(mc + 1) * 128],
                        ident[0:65, 0:65])
                    den = fin.tile([128, 1], f32, tag=f"den{j}{mc}")
                    nc.vector.reciprocal(den[:], pt[:, 0, 64:65])
                    kve = fin.tile([128, 64], f32, tag=f"kve{j}{mc}")
                    esum = fin.tile([128, 1], f32, tag=f"es{j}{mc}")
                    nc.scalar.activation(kve[:], pt[:, 0, 0:64], EXP,
                                         scale=den[:], accum_out=esum[:])
                    rsum = fin.tile([128, 1], f32, tag=f"rs{j}{mc}")
                    nc.vector.reciprocal(rsum[:], esum[:])
                    nc.vector.tensor_scalar_mul(kv_aug[j][mc][:, 0:64],
                                                kve[:], rsum[:])
                    nc.vector.tensor_copy(kv_aug[j][mc][:, 64:66],
                                          ones_h[:, 0:2])

        # ================= PASS B: out = (e1/rowsum) @ kv =================
        with ExitStack() as bctx:
            outp = bctx.enter_context(tc.tile_pool(name="outp", bufs=4))
            pout = bctx.enter_context(tc.tile_pool(name="pout", bufs=4, space="PSUM"))

            # 2 token-tiles (256 tokens) per 2-bank psum tile to halve the
            # per-iteration slot/sem overhead.
            for it in range(N // 256):
                c0 = it * 256
                # padded so the u-dim stride is one full psum bank: head j=3's
                # 264B output must not cross the 2KB bank boundary.
                po = pout.tile([128, 2, 4, 66], f32, tag="pout",
                               padded_shape=[128, 2, 4, 128])
                for u in range(2):
                    for hp in range(2):
                        for jj in range(2):
                            j = 2 * hp + jj
                            for mc in range(2):
                                # col-tiled concurrent 64-token pair;
                                # pending-zero marks are per-partition, so
                                # each half needs its own start/stop.
                                for half in range(2):
                                    t0 = c0 + u * 128 + half * 64
                                    nc.tensor.matmul(
                                        po[half * 64:(half + 1) * 64, u, j, :],
                                        e1s[hp][mc][:, jj, t0:t0 + 64],
                                        kv_aug[j][mc][:],
                                        start=(j == 0 and mc == 0),
                                        stop=(j == 3 and mc == 1))
                rec = outp.tile([128, 2, 4], f32, tag="rec")
                nc.vector.reciprocal(rec[:], po[:, :, :, 64])
                ot = outp.tile([128, 2, 4, 64], f32, tag="ot")
                nc.vector.tensor_tensor(
                    ot[:], po[:, :, :, 0:64],
                    rec[:].unsqueeze(3).broadcast_to((128, 2, 4, 64)),
                    mybir.AluOpType.mult)
                nc.sync.dma_start(
                    out_ap[c0:c0 + 256, :].rearrange("(u p) c -> p u c", u=2),
                    ot[:].rearrange("p u j d -> p u (j d)"))

    nc.compile()
    return nc


def _get_program():
    if "nc" not in _cached:
        _cached["nc"] = _build()
    return _cached["nc"]


def kernel(x, w_qkv, agent):
    from concourse.bass_utils import run_bass_kernel_spmd

    nc = _get_program()

    x = np.asarray(x, dtype=np.float32)
    w_qkv = np.asarray(w_qkv, dtype=np.float32)
    agent = np.asarray(agent, dtype=np.float32)

    in_maps = []
    for core in range(8):
        bi, hg = core // 2, core % 2
        heads = [4 * hg + jj for jj in range(4)]
        wqk = np.empty((DIM, 512), np.float16)
        for hp in range(2):
            hA, hB = heads[2 * hp], heads[2 * hp + 1]
            wqk[:, hp * 256 + 0:hp * 256 + 64] = w_qkv[hA * 64:(hA + 1) * 64, :].T
            wqk[:, hp * 256 + 64:hp * 256 + 128] = w_qkv[hB * 64:(hB + 1) * 64, :].T
            wqk[:, hp * 256 + 128:hp * 256 + 192] = \
                w_qkv[DIM + hA * 64:DIM + (hA + 1) * 64, :].T
            wqk[:, hp * 256 + 192:hp * 256 + 256] = \
                w_qkv[DIM + hB * 64:DIM + (hB + 1) * 64, :].T
        # pack [512c, 512cols] -> [128p, 4ci*512cols]
        wqk_p = np.ascontiguousarray(
            wqk.reshape(4, 128, 512).transpose(1, 0, 2).reshape(128, 2048))
        wv = np.empty((DIM, 256), np.float16)
        for jj, hh in enumerate(heads):
            wv[:, jj * 64:(jj + 1) * 64] = \
                w_qkv[2 * DIM + hh * 64:2 * DIM + (hh + 1) * 64, :].T
        wv_p = np.ascontiguousarray(
            wv.reshape(4, 128, 256).transpose(1, 0, 2).reshape(128, 1024))
        ag = np.empty((128, 1024), np.float16)
        for jj, hh in enumerate(heads):
            agT = agent[hh].T
            ag[0:64, jj * 256:(jj + 1) * 256] = agT
            ag[64:128, jj * 256:(jj + 1) * 256] = agT
        # x: [n, 512c] -> xT [512c, n] -> packed [(st p), (ci t)]
        xt = x[bi].T.astype(np.float16)                       # [512, 8192]
        xt_p = np.ascontiguousarray(
            xt.reshape(4, 128, NST, ST).transpose(2, 1, 0, 3)
            .reshape(NST * 128, 4 * ST))
        in_maps.append({"x": xt_p, "wqk": wqk_p, "wv": wv_p, "ag": ag})

    res = run_bass_kernel_spmd(nc, in_maps, core_ids=list(range(8)),
                               trace=bool(os.environ.get("AGENT_TRACE")))
    out = np.empty((B, N, DIM), np.float32)
    for core in range(8):
        bi, hg = core // 2, core % 2
        out[bi, :, hg * 256:(hg + 1) * 256] = res.results[core]["out"]
    if res.exec_time_ns is not None:
        kernel.last_exec_time_ns = res.exec_time_ns
        kernel.last_mean_exec_time_ns = res.mean_exec_time_ns
        kernel.last_trace = res.instructions_and_trace
    return out
